# revision 30
# baseline (speedup 1.0000x reference)
"""2-layer GAT encoder on 8 Trainium2 NeuronCores.

Sharding: destination nodes (and their incoming edges) are partitioned across
the 8 cores (6250 dst nodes each).  Edges are sorted by dst on the host,
grouped into windows of 128 dst nodes, split into a low-src / high-src run
(int16 gather indices), and padded to a fixed tile count so every core runs
the identical SPMD program.

Four device launches; between launches the host does only index-space work
(shard/concat/transpose, and per-edge fancy-indexing of the tiny per-node
attention-coefficient tables into streamed per-edge arrays):

  A: xp1 = x @ W1ext        (per-core node shard)  -> feature rows + a_src/a_dst
  B: layer-1 edge phase     (gather + segment softmax reduction) -> h shard
  C: xp2 = h @ W2ext        (per-core node shard)
  D: layer-2 edge phase     -> out shard

The edge phase gathers 512B source-feature rows per edge with dma_gather,
builds a weighted one-hot per 128-edge tile (one tensor_scalar is_equal*mult
per head) and segment-reduces via PSUM-accumulating matmuls; denominators come
from ones-column matmuls; the epilogue divides, adds bias and applies ELU.
"""

import numpy as np
from dataclasses import dataclass

import concourse.bass as bass
import concourse.bacc as bacc
import concourse.tile as tile
import concourse.mybir as mybir
from concourse.bass_utils import run_bass_kernel_spmd

AF = mybir.ActivationFunctionType
ALU = mybir.AluOpType
F32 = mybir.dt.float32
I16 = mybir.dt.int16

SPLIT = 32768  # int16 gather-index limit -> low/high table split


@dataclass
class GatCfg:
    n: int = 50000
    d_in: int = 128
    c1: int = 64
    c2: int = 32
    n_cores: int = 8
    wwin: int = 128
    gchunk: int = 8   # tiles per dma_gather (HW SWDGE ring holds ~1024 descs)

    @property
    def ns(self):
        return self.n // self.n_cores

    @property
    def nwin(self):
        return (self.ns + self.wwin - 1) // self.wwin


def _wrap_idx(idx):
    """[num] int32 -> [128, num/16] int16 dma_gather layout (16-wrap, x8)."""
    num = idx.shape[0]
    assert num % 16 == 0
    w = idx.reshape(num // 16, 16).T.astype(np.int16)   # [16, num/16]
    return np.ascontiguousarray(np.tile(w, (8, 1)))      # [128, num/16]


def _host_prep(cfg: GatCfg, edge_index):
    """Shard + window + split(low/high src) + pad the edge list.

    Returns (TwA, TwB, gixA, gixB, dloc, esrc, edst) where gix* are the
    wrapped int16 gather indices [ncore, nwin, 128, Tw*8], dloc the in-window
    dst slot (-1 padding) [ncore, nwin, 128, Tw], and esrc/edst the absolute
    src/dst node ids per slot [ncore, nwin, 128, Tw] (padding -> node 0).
    """
    n, ns, wwin, nwin = cfg.n, cfg.ns, cfg.wwin, cfg.nwin
    loops = np.arange(n, dtype=np.int64)
    src = np.concatenate([edge_index[0].astype(np.int64), loops])
    dst = np.concatenate([edge_index[1].astype(np.int64), loops])
    # group by (core, window); within a window: low-src run then high-src run
    gwin_raw = (dst // ns) * nwin + (dst % ns) // wwin
    order = np.lexsort((src >= SPLIT, gwin_raw))
    src, dst = src[order], dst[order]
    gwin = gwin_raw[order]
    low = src < SPLIT
    ngrp = cfg.n_cores * nwin
    cnt_lo = np.bincount(gwin[low], minlength=ngrp)
    cnt_hi = np.bincount(gwin[~low], minlength=ngrp)
    TwA = int(np.ceil(cnt_lo.max() / 128))
    TwB = int(np.ceil(cnt_hi.max() / 128))
    Tw = TwA + TwB

    gidx = np.zeros((cfg.n_cores, nwin, Tw * 128), dtype=np.int32)
    dloc = np.full((cfg.n_cores, nwin, Tw * 128), -1.0, dtype=np.float32)
    esrc = np.zeros((cfg.n_cores, nwin, Tw * 128), dtype=np.int32)
    edst = np.zeros((cfg.n_cores, nwin, Tw * 128), dtype=np.int32)

    starts_all = np.concatenate([[0], np.cumsum(np.bincount(gwin, minlength=ngrp))])
    for c in range(cfg.n_cores):
        for w in range(nwin):
            g = c * nwin + w
            s0, s1 = starts_all[g], starts_all[g + 1]
            nlo = cnt_lo[g]
            # group slice is low-run then high-run (lexsort secondary key)
            for base, a, b, off in ((0, s0, s0 + nlo, 0),
                                    (TwA * 128, s0 + nlo, s1, SPLIT)):
                m = b - a
                if m == 0:
                    continue
                sl = slice(base, base + m)
                gidx[c, w, sl] = src[a:b] - off
                dloc[c, w, sl] = (dst[a:b] % ns - w * wwin).astype(np.float32)
                esrc[c, w, sl] = src[a:b]
                edst[c, w, sl] = dst[a:b]

    # wrap gather indices; reshape slot arrays to [.., 128, Tw] ([w,p,t], j=t*128+p)
    gixA = np.zeros((cfg.n_cores, nwin, 128, TwA * 8), dtype=np.int16)
    gixB = np.zeros((cfg.n_cores, nwin, 128, TwB * 8), dtype=np.int16)
    for c in range(cfg.n_cores):
        for w in range(nwin):
            gixA[c, w] = _wrap_idx(gidx[c, w, :TwA * 128])
            gixB[c, w] = _wrap_idx(gidx[c, w, TwA * 128:])

    def slots(arr):
        # [.., Tw*128] (j = t*128+p) -> [.., 128, Tw]
        return np.ascontiguousarray(
            arr.reshape(cfg.n_cores, nwin, Tw, 128).transpose(0, 1, 3, 2))

    return TwA, TwB, slots(dloc), slots(esrc), slots(edst), gixA, gixB


# --------------------------------------------------------------------------
# launch builders
# --------------------------------------------------------------------------

def build_table_kernel(nc, cfg: GatCfg, feat2, name):
    """xp = xT_shard.T @ Wext; emits feature rows + al_src + al_dst tables.

    feat2: total feature columns (2*c). Wext has feat2+4 columns.
    """
    ns = cfg.ns
    wcols = feat2 + 4
    xT = nc.dram_tensor("xT", [cfg.d_in, ns], F32, kind="ExternalInput")
    we = nc.dram_tensor("we", [cfg.d_in, wcols], F32, kind="ExternalInput")
    xp = nc.dram_tensor("xp", [ns, feat2], F32, kind="ExternalOutput")
    als = nc.dram_tensor("als", [ns, 2], F32, kind="ExternalOutput")
    ald = nc.dram_tensor("ald", [ns, 2], F32, kind="ExternalOutput")

    with tile.TileContext(nc) as tc:
        with (
            tc.tile_pool(name="c", bufs=1) as cpool,
            tc.tile_pool(name="x", bufs=3) as xpool,
            tc.tile_pool(name="ps", bufs=8, space="PSUM") as pspool,
            tc.tile_pool(name="o", bufs=8) as opool,
        ):
            ws = cpool.tile([128, wcols], F32)
            nc.sync.dma_start(ws[:], we.ap()[:, :])
            BLK = 2048
            k = 0
            for nb0 in range(0, ns, BLK):
                bsz = min(BLK, ns - nb0)
                xt = xpool.tile([128, BLK], F32, tag="xt")
                nc.sync.dma_start(xt[:, :bsz], xT.ap()[:, nb0:nb0 + bsz])
                for j in range(0, bsz, 128):
                    m = min(128, bsz - j)
                    ps = pspool.tile([128, wcols], F32, tag="ps")
                    nc.tensor.matmul(ps[:m, :], xt[:, j:j + m], ws[:],
                                     start=True, stop=True)
                    ob = opool.tile([128, wcols], F32, tag="ob")
                    if k % 2 == 0:
                        nc.vector.tensor_copy(ob[:m, :], ps[:m, :])
                    else:
                        nc.scalar.copy(ob[:m, :], ps[:m, :])
                    k += 1
                    r0 = nb0 + j
                    nc.sync.dma_start(xp.ap()[r0:r0 + m, :], ob[:m, 0:feat2])
                    nc.sync.dma_start(als.ap()[r0:r0 + m, :],
                                      ob[:m, feat2:feat2 + 2])
                    nc.sync.dma_start(ald.ap()[r0:r0 + m, :],
                                      ob[:m, feat2 + 2:feat2 + 4])
    return nc


def build_edge_kernel(nc, cfg: GatCfg, TwA, TwB, cdim, out_cols, name,
                      dbg=False):
    """Edge phase for one layer.  cdim = per-head dim (64 / 32)."""
    ns, nwin, wwin = cfg.ns, cfg.nwin, cfg.wwin
    Tw = TwA + TwB
    feat2 = 2 * cdim
    if dbg:
        dbg_X = nc.dram_tensor("dbg_X", [128, Tw * feat2], F32,
                               kind="ExternalOutput")
        dbg_wt = nc.dram_tensor("dbg_wt", [128, Tw * 2], F32,
                                kind="ExternalOutput")
        dbg_m0 = nc.dram_tensor("dbg_m0", [128, wwin], F32,
                                kind="ExternalOutput")
        dbg_ps = nc.dram_tensor("dbg_ps", [128, 2 * (cdim + 2)], F32,
                                kind="ExternalOutput")

    xp = nc.dram_tensor("xp", [cfg.n, feat2], F32, kind="ExternalInput")
    gixA = nc.dram_tensor("gixA", [nwin, 128, TwA * 8], I16, kind="ExternalInput")
    gixB = nc.dram_tensor("gixB", [nwin, 128, TwB * 8], I16, kind="ExternalInput")
    dlc = nc.dram_tensor("dlc", [nwin, 128, Tw], F32, kind="ExternalInput")
    alsE = nc.dram_tensor("alsE", [nwin, 128, Tw, 2], F32, kind="ExternalInput")
    aldE = nc.dram_tensor("aldE", [nwin, 128, Tw, 2], F32, kind="ExternalInput")
    bb = nc.dram_tensor("bb", [128, out_cols], F32, kind="ExternalInput")
    iot = nc.dram_tensor("iot", [128, wwin], F32, kind="ExternalInput")
    out = nc.dram_tensor("out", [ns, out_cols], F32, kind="ExternalOutput")

    with tile.TileContext(nc) as tc:
        with (
            tc.tile_pool(name="c", bufs=1) as cpool,
            tc.tile_pool(name="i", bufs=3) as ipool,
            tc.tile_pool(name="x", bufs=2) as xpool,
            tc.tile_pool(name="w", bufs=2) as wpool,
            tc.tile_pool(name="m", bufs=4) as mpool,
            tc.tile_pool(name="ps", bufs=2, space="PSUM") as pspool,
            tc.tile_pool(name="e", bufs=2) as epool,
        ):
            bs = cpool.tile([128, out_cols], F32)
            nc.sync.dma_start(bs[:], bb.ap()[:, :])
            ios = cpool.tile([128, wwin], F32)
            nc.sync.dma_start(ios[:], iot.ap()[:, :])
            ones2 = cpool.tile([128, 2], F32)
            nc.vector.memset(ones2[:], 1.0)

            for w in range(nwin):
                gA = ipool.tile([128, TwA * 8], I16, tag="gA")
                nc.sync.dma_start(gA[:], gixA.ap()[w])
                gB = ipool.tile([128, TwB * 8], I16, tag="gB")
                nc.sync.dma_start(gB[:], gixB.ap()[w])
                dl = ipool.tile([128, Tw], F32, tag="dl")
                nc.sync.dma_start(dl[:], dlc.ap()[w])
                sv = wpool.tile([128, Tw, 2], F32, tag="sv")
                nc.sync.dma_start(sv[:], alsE.ap()[w])
                ad = ipool.tile([128, Tw, 2], F32, tag="ad")
                nc.sync.dma_start(ad[:], aldE.ap()[w])

                X = xpool.tile([128, Tw, feat2], F32, tag="X")
                for t0_, nt_, gi_, tab in (
                    (0, TwA, gA, xp.ap()[0:SPLIT, :]),
                    (TwA, TwB, gB, xp.ap()[SPLIT:cfg.n, :]),
                ):
                    for cb in range(0, nt_, cfg.gchunk):
                        ct = min(cfg.gchunk, nt_ - cb)
                        nc.gpsimd.dma_gather(
                            X[:, t0_ + cb:t0_ + cb + ct, :], tab,
                            gi_[:, cb * 8:(cb + ct) * 8],
                            num_idxs=ct * 128, num_idxs_reg=ct * 128,
                            elem_size=feat2)

                # w = exp(leakyrelu(al_src + al_dst, 0.2))
                nc.vector.tensor_tensor(sv[:], sv[:], ad[:], ALU.add)
                s2 = wpool.tile([128, Tw, 2], F32, tag="s2")
                nc.vector.tensor_scalar(s2[:], sv[:], 0.2, None, ALU.mult)
                nc.vector.tensor_tensor(sv[:], sv[:], s2[:], ALU.max)
                wt = wpool.tile([128, Tw, 2], F32, tag="wt")
                nc.scalar.activation(wt[:], sv[:], AF.Exp)

                if dbg and w == 0:
                    nc.sync.dma_start(dbg_X.ap()[:, :],
                                      X[:].rearrange("p a b -> p (a b)"))
                    nc.sync.dma_start(dbg_wt.ap()[:, :],
                                      wt[:].rearrange("p a b -> p (a b)"))

                ps0 = pspool.tile([128, cdim], F32, tag="ps0")
                ps1 = pspool.tile([128, cdim], F32, tag="ps1")
                pd0 = pspool.tile([128, 2], F32, tag="pd0")
                pd1 = pspool.tile([128, 2], F32, tag="pd1")
                for t in range(Tw):
                    st, sp = (t == 0), (t == Tw - 1)
                    m0 = mpool.tile([128, wwin], F32, tag="m0")
                    nc.vector.tensor_scalar(
                        m0[:], ios[:, :], dl[:, t:t + 1], wt[:, t, 0:1],
                        ALU.is_equal, ALU.mult)
                    if dbg and w == 0 and t == 0:
                        nc.sync.dma_start(dbg_m0.ap()[:, :], m0[:])
                    nc.tensor.matmul(ps0[:, :], m0[:], X[:, t, 0:cdim],
                                     start=st, stop=sp)
                    nc.tensor.matmul(pd0[:, :], m0[:], ones2[:],
                                     start=st, stop=sp)
                    m1 = mpool.tile([128, wwin], F32, tag="m1")
                    nc.vector.tensor_scalar(
                        m1[:], ios[:, :], dl[:, t:t + 1], wt[:, t, 1:2],
                        ALU.is_equal, ALU.mult)
                    nc.tensor.matmul(ps1[:, :], m1[:], X[:, t, cdim:feat2],
                                     start=st, stop=sp)
                    nc.tensor.matmul(pd1[:, :], m1[:], ones2[:],
                                     start=st, stop=sp)

                # epilogue: divide, +bias, ELU, store rows
                wd = min(wwin, ns - w * wwin)
                if dbg and w == 0:
                    pdb_ = epool.tile([128, 2 * (cdim + 2)], F32, tag="pdb")
                    nc.vector.tensor_copy(pdb_[:, 0:cdim], ps0[:, :])
                    nc.vector.tensor_copy(pdb_[:, cdim:cdim + 2], pd0[:, :])
                    nc.vector.tensor_copy(pdb_[:, cdim + 2:2 * cdim + 2], ps1[:, :])
                    nc.vector.tensor_copy(pdb_[:, 2 * cdim + 2:], pd1[:, :])
                    nc.sync.dma_start(dbg_ps.ap()[:, :], pdb_[:])
                den = epool.tile([128, 2], F32, tag="den")
                nc.vector.tensor_scalar(den[:, 0:1], pd0[:, 0:1],
                                        1e-30, None, ALU.max)
                nc.vector.tensor_scalar(den[:, 1:2], pd1[:, 0:1],
                                        1e-30, None, ALU.max)
                rcp = epool.tile([128, 2], F32, tag="rcp")
                nc.vector.reciprocal(rcp[:], den[:])
                V = epool.tile([128, out_cols], F32, tag="V")
                nc.vector.tensor_scalar(V[:, 0:cdim], ps0[:, :],
                                        rcp[:, 0:1], None, ALU.mult)
                nc.vector.tensor_scalar(V[:, cdim:feat2], ps1[:, :],
                                        rcp[:, 1:2], None, ALU.mult)
                nc.vector.tensor_tensor(V[:], V[:], bs[:], ALU.add)
                E = epool.tile([128, out_cols], F32, tag="E")
                nc.vector.tensor_scalar(E[:], V[:], 0.0, None, ALU.min)
                nc.scalar.activation(E[:], E[:], AF.Exp)
                nc.vector.tensor_scalar(E[:], E[:], -1.0, None, ALU.add)
                H = epool.tile([128, out_cols], F32, tag="H")
                nc.vector.tensor_tensor(H[:], V[:], E[:], ALU.max)
                nc.sync.dma_start(out.ap()[w * wwin:w * wwin + wd, :], H[:wd, :])
    return nc


# --------------------------------------------------------------------------
# host orchestration
# --------------------------------------------------------------------------

def _ext_w(W, a_s, a_d, c):
    """[d, 2c+4] = [W | W_lo@a_s0 | W_hi@a_s1 | W_lo@a_d0 | W_hi@a_d1]."""
    return np.ascontiguousarray(np.concatenate([
        W,
        (W[:, :c] @ a_s[0])[:, None], (W[:, c:] @ a_s[1])[:, None],
        (W[:, :c] @ a_d[0])[:, None], (W[:, c:] @ a_d[1])[:, None],
    ], axis=1), dtype=np.float32)


SIM_MODE = False  # set True to run launches in CoreSim instead of hardware


class _SimRes:
    def __init__(self, results):
        self.results = results
        self.exec_time_ns = None


class _Launch:
    def __init__(self, nc, cfg):
        self.nc = nc
        self.cfg = cfg

    def run(self, in_maps, trace=False):
        if SIM_MODE:
            from concourse.bass_interp import MultiCoreSim
            sim = MultiCoreSim(self.nc, num_cores=self.cfg.n_cores, trace=False,
                               require_finite=False, require_nnan=False)
            cores = list(sim.cores.values())
            for c, core in enumerate(cores):
                for k, v in in_maps[c].items():
                    core.tensor(k)[:] = v
            sim.simulate(check_with_hw=False)
            outs = []
            for core in cores:
                d = {}
                for alloc in self.nc.m.functions[0].allocations:
                    if (isinstance(alloc, mybir.MemoryLocationSet)
                            and alloc.kind == "ExternalOutput"):
                        nm = alloc.memorylocations[0].name
                        d[nm] = np.array(core.tensor(nm))
                outs.append(d)
            return _SimRes(outs)
        res = run_bass_kernel_spmd(self.nc, in_maps,
                                   core_ids=list(range(self.cfg.n_cores)),
                                   trace=trace)
        return res


def prepare(x, edge_index, W1, a_src1, a_dst1, b1, W2, a_src2, a_dst2, b2,
            cfg=None):
    x = np.asarray(x, dtype=np.float32)
    cfg = cfg or GatCfg()
    TwA, TwB, dloc, esrc, edst, gixA, gixB = _host_prep(
        cfg, np.asarray(edge_index))
    Tw = TwA + TwB
    ncore, ns, nwin = cfg.n_cores, cfg.ns, cfg.nwin

    w1e = _ext_w(np.asarray(W1, np.float32), np.asarray(a_src1, np.float32),
                 np.asarray(a_dst1, np.float32), cfg.c1)
    w2e = _ext_w(np.asarray(W2, np.float32), np.asarray(a_src2, np.float32),
                 np.asarray(a_dst2, np.float32), cfg.c2)
    b1b = np.ascontiguousarray(
        np.broadcast_to(np.asarray(b1, np.float32)[None, :], (128, 2 * cfg.c1)))
    b2b = np.ascontiguousarray(
        np.broadcast_to(np.asarray(b2, np.float32)[None, :], (128, 2 * cfg.c2)))
    iot = np.ascontiguousarray(
        np.broadcast_to(np.arange(cfg.wwin, dtype=np.float32)[None, :],
                        (128, cfg.wwin)))

    # ---- build + compile the four programs ----
    def mk(builder, *args):
        nc = bacc.Bacc("TRN2", num_devices=ncore, debug=False)
        builder(nc, *args)
        nc.compile()
        return _Launch(nc, cfg)

    LA = mk(build_table_kernel, cfg, 2 * cfg.c1, "t1")
    LB = mk(build_edge_kernel, cfg, TwA, TwB, cfg.c1, 2 * cfg.c1, "e1")
    LC = mk(build_table_kernel, cfg, 2 * cfg.c2, "t2")
    LD = mk(build_edge_kernel, cfg, TwA, TwB, cfg.c2, 2 * cfg.c2, "e2")

    def run_all(trace=False):
        exec_ns = []

        def _t(res):
            if res.exec_time_ns is not None:
                exec_ns.append(res.exec_time_ns)

        # A: layer-1 tables (node-sharded)
        inA = [{"xT": np.ascontiguousarray(x[c * ns:(c + 1) * ns].T),
                "we": w1e} for c in range(ncore)]
        rA = LA.run(inA, trace)
        _t(rA)
        xp1 = np.concatenate([rA.results[c]["xp"] for c in range(ncore)])
        als1 = np.concatenate([rA.results[c]["als"] for c in range(ncore)])
        ald1 = np.concatenate([rA.results[c]["ald"] for c in range(ncore)])

        # B: layer-1 edge phase
        inB = [{"xp": xp1,
                "gixA": gixA[c], "gixB": gixB[c],
                "dlc": dloc[c],
                "alsE": np.ascontiguousarray(als1[esrc[c]]),
                "aldE": np.ascontiguousarray(ald1[edst[c]]),
                "bb": b1b, "iot": iot} for c in range(ncore)]
        rB = LB.run(inB, trace)
        _t(rB)
        h = np.concatenate([rB.results[c]["out"] for c in range(ncore)])

        # C: layer-2 tables
        inC = [{"xT": np.ascontiguousarray(h[c * ns:(c + 1) * ns].T),
                "we": w2e} for c in range(ncore)]
        rC = LC.run(inC, trace)
        _t(rC)
        xp2 = np.concatenate([rC.results[c]["xp"] for c in range(ncore)])
        als2 = np.concatenate([rC.results[c]["als"] for c in range(ncore)])
        ald2 = np.concatenate([rC.results[c]["ald"] for c in range(ncore)])

        # D: layer-2 edge phase
        inD = [{"xp": xp2,
                "gixA": gixA[c], "gixB": gixB[c],
                "dlc": dloc[c],
                "alsE": np.ascontiguousarray(als2[esrc[c]]),
                "aldE": np.ascontiguousarray(ald2[edst[c]]),
                "bb": b2b, "iot": iot} for c in range(ncore)]
        rD = LD.run(inD, trace)
        _t(rD)
        out = np.concatenate([rD.results[c]["out"] for c in range(ncore)])
        total_ns = sum(exec_ns) if len(exec_ns) == 4 else None
        return out, total_ns

    return cfg, (TwA, TwB), run_all


def kernel(x, edge_index, W1, a_src1, a_dst1, b1, W2, a_src2, a_dst2, b2):
    x = np.asarray(x, dtype=np.float32)
    _, _, run_all = prepare(x, edge_index, W1, a_src1, a_dst1, b1,
                            W2, a_src2, a_dst2, b2)
    out, _ = run_all()
    return out, x
